# revision 2
# baseline (speedup 1.0000x reference)
"""Canny NMS filter for 8x Trainium2 NeuronCores (Bass/Tile) — v2.

Batch sharded one image per core. Per core (img 3x1024x1024 -> 1024x1024):

  m   = c0+c1+c2          gpsimd DMA-accumulate loads, cast f32->f32r
  mb  = gauss3x3 (x) m    3 banded fp32r matmuls per 512-chunk (PE)
  gx/gy = sobel (x) mb    2+3 banded fp32r matmuls per chunk (PE)
  sx/sy = Square(psum)    ACT
  t   = recip(gx)*gy      DVE (exact); th = Arctan(t) on ACT (~1e-7)
  orientation: keep_ori <=> | |th*8/pi - 1| - 2 | >= 0.5  (TS chain with
     2 exact bitwise-AND abs) -- reproduces round-half-even boundaries
  M = sx+sy (Pool, fp32r); z = M + B*keep_ori (DVE, fp32r)
  pul = ul(x)M - I(x)z ; pdr = dr(x)M - I(x)z    fp32r shift matmuls (PE)
  s1 = Sign(-pul), s2 = Sign(-pdr) on ACT; keep <=> s1+s2 == 2
  out = M * keep (fp16), stored fp16; host converts to fp32

Orientation algebra: oriented <=> round(arctan(t)*8/pi + 4) % 8 in {3,7}
with jnp.round's round-half-even <=> u = arctan(t)*8/pi - 1 satisfies
||u|-2| < 0.5 (all four half-integer boundaries land on the inclusive
side of >=, matching banker's rounding). NMS row clamp at the image top/
bottom is folded into the shift bands; column clamp via 1-col edge pads.

Row axis: 9 slabs of 128 partitions (122 out rows each, image row
r = 122*s - 3 + p); sobel/shift bands clamp at image edges (variants).
Emission is software-pipelined: slab s+1's loads are issued before slab
s's compute so the in-order SEQs always have ready work.
"""

import math
import numpy as np

B, C, H, W = 8, 3, 1024, 1024
NCORES = 8
SLAB = 122
NSLABS = (H + SLAB - 1) // SLAB          # 9
FW = W + 2                                # col c stored at f = c + 1
CHUNK = 512
BIGV = 8192.0                             # exact in fp16/fp32r

_CACHE = {}


# ---------------------------------------------------------------------------
def _install_fixups():
    """This container's walrus encodes at most ONE sem wait per instruction
    (2 for EventSemaphore); the bass/tile build attaches more. Two patches:
    the TileContext tail drain (waits on every proc's clock) is split into a
    chain of single-wait sync nops, and a post-schedule pass moves excess
    waits from any instruction onto injected same-engine NoOps."""
    import concourse.tile as _tile
    from concourse.vector_clock import ScopedClock, VectorClock

    if getattr(_tile.TileContext, "_canny_patched", False):
        return

    def _drain_and_barrier(self, tick_clock, wait_clock):
        gcl = tick_clock.global_clock
        for i in range(len(gcl)):
            if gcl[i] == 0:
                continue
            vec = [0] * len(gcl)
            vec[i] = gcl[i]
            nop = self.nc.sync.nop(nofuse=True, hint="tail_drain_split")
            wait_clock.add_sem_waits(nop.ins,
                                     ScopedClock({None: VectorClock(vec)}))
        self.nc.sync.drain()
        self.nc.all_engine_barrier()
        assert self.sems is not None
        popped = self.nc._tile_sem_poison_stack.pop()
        assert popped is self._sem_poison
        self.nc.clear_and_free_semaphores(list(self.sems.allocated().values()))
        self.nc.all_engine_barrier()

    _tile.TileContext._drain_and_barrier = _drain_and_barrier
    _tile.TileContext._canny_patched = True


def _split_excess_waits(nc):
    import concourse.mybir as mybir
    for fn in nc.m.functions:
        for blk in fn.blocks:
            insts = list(blk.instructions)
            out, changed = [], False
            for inst in insts:
                si = inst.sync_info
                cap = 2 if isinstance(inst, mybir.InstEventSemaphore) else 1
                if si is not None and si.on_wait and len(si.on_wait) > cap:
                    waits = list(si.on_wait)
                    for j, wt in enumerate(waits[cap:]):
                        nop = mybir.InstNoOp(name=f"{inst.name}-wsplit{j}")
                        nop.engine = inst.engine
                        nop.sync_info = mybir.SyncInfo(on_wait=[wt],
                                                       on_update=[])
                        out.append(nop)
                    si.on_wait = waits[:cap]
                    inst.sync_info = si
                    changed = True
                out.append(inst)
            if changed:
                blk.instructions = out


# ---------------------------------------------------------------------------
# host-side band-matrix construction
def _r0(s):
    return SLAB * s - 3


def _band(s, taps, clamp):
    """lhsT[k, m]: out[m] = sum_j taps[j] * in[k(m, j)] for slab s.
    k(m, j) = m + j, optionally clamped (in partition space) to the image
    edge partitions; unclamped out-of-range taps are dropped."""
    Wm = np.zeros((128, 128), np.float64)
    lo = 3 if (clamp and s == 0) else None
    hi = (H - 1 - _r0(s)) if (clamp and s == NSLABS - 1) else None
    for m in range(128):
        for off, cf in taps.items():
            k = m + off
            if lo is not None and k < lo:
                k = lo
            if hi is not None and k > hi:
                k = hi
            if 0 <= k < 128:
                Wm[k, m] += cf
    return Wm


def _col_taps(k3x3, dc):
    col = k3x3[:, dc]
    return {j - 1: col[j] for j in range(3)}


def _build_bands(gauss_w, sobel_x, sobel_y, dir_w):
    """wf f32 (fp32r conv weights), wh f16 (NMS shift weights);
    index: conv keys -> int i into wf; NMS keys -> ("h", i) into wh.
    Variants v: 0 = top slab, 1 = interior, 2 = bottom slab."""
    g = np.asarray(gauss_w, np.float64).reshape(3, 3) / 3.0
    sx = np.asarray(sobel_x, np.float64).reshape(3, 3)
    sy = np.asarray(sobel_y, np.float64).reshape(3, 3)
    dw = np.asarray(dir_w, np.float64).reshape(8, 3, 3)

    wf, index = [], {}

    def add(key, mat64):
        index[key] = len(wf)
        wf.append(np.asarray(mat64, np.float32))

    # blur: no clamping (m carries duplicated edge rows)
    for dc in range(3):
        add(("blur", dc), _band(4, _col_taps(g, dc), clamp=False))
    # sobel: clamp to image-edge partitions
    for v in range(3):
        s = {0: 0, 1: 4, 2: NSLABS - 1}[v]
        for nm, kk in (("gx", sx), ("gy", sy)):
            for dc in range(3):
                if not np.any(kk[:, dc]):
                    continue
                add((nm, v, dc), _band(s, _col_taps(kk, dc), clamp=True))
    # NMS shift bands (fp16): from dir_w channels 3 and 7
    wh = []

    def addh(key, mat64):
        index[key] = ("h", len(wh))
        wh.append(np.asarray(mat64, np.float32))

    delta = np.zeros((3, 3))
    delta[1, 1] = 1.0
    for v in range(3):
        s = {0: 0, 1: 4, 2: NSLABS - 1}[v]
        for nm, kk in (("ul", delta - dw[3]), ("dr", delta - dw[7])):
            for dc in range(3):
                if not np.any(kk[:, dc]):
                    continue
                addh((nm, v, dc), _band(s, _col_taps(kk, dc), clamp=True))
    addh(("negI",), -np.eye(128, dtype=np.float32))
    wf = np.stack(wf).astype(np.float32)
    wh = np.stack(wh).astype(np.float32).astype(np.float16)
    return wf, wh, index


def _structure_key(index):
    return tuple(sorted(map(repr, index.keys())))


# ---------------------------------------------------------------------------
def _build_module(index, nf, nh):
    import concourse.bass as bass
    import concourse.tile as tile
    import concourse.mybir as mybir
    from contextlib import ExitStack

    F32 = mybir.dt.float32
    F32R = mybir.dt.float32r
    F16 = mybir.dt.float16
    U32 = mybir.dt.uint32
    AF = mybir.ActivationFunctionType
    Al = mybir.AluOpType

    nc = bass.Bass("TRN2", target_bir_lowering=False, debug=False,
                   num_devices=NCORES)
    img_d = nc.dram_tensor("img", [C, H, W], F32, kind="ExternalInput").ap()
    wf_d = nc.dram_tensor("wf", [nf, 128, 128], F32R,
                          kind="ExternalInput").ap()
    wh_d = nc.dram_tensor("wh", [nh, 128, 128], F16,
                          kind="ExternalInput").ap()
    out_d = nc.dram_tensor("out", [H, W], F16, kind="ExternalOutput").ap()

    with tile.TileContext(nc) as tc, ExitStack() as ctx:
        wpool = ctx.enter_context(tc.tile_pool(name="wpool", bufs=1))
        mpool = ctx.enter_context(tc.tile_pool(name="mpool", bufs=3))
        mbp = ctx.enter_context(tc.tile_pool(name="mbp", bufs=2))
        sqp = ctx.enter_context(tc.tile_pool(name="sqp", bufs=2))
        tp = ctx.enter_context(tc.tile_pool(name="tp", bufs=2))
        mzp = ctx.enter_context(tc.tile_pool(name="mzp", bufs=2))
        klp = ctx.enter_context(tc.tile_pool(name="klp", bufs=2))
        ps_b = ctx.enter_context(
            tc.tile_pool(name="ps_b", bufs=2, space="PSUM"))
        ps_g = ctx.enter_context(
            tc.tile_pool(name="ps_g", bufs=2, space="PSUM"))
        ps_n = ctx.enter_context(
            tc.tile_pool(name="ps_n", bufs=2, space="PSUM"))

        wft = wpool.tile([128, nf * 128], F32R, name="wft")
        nc.sync.dma_start(wft[:].rearrange("k (n m) -> k n m", n=nf),
                          wf_d.rearrange("n k m -> k n m"))
        wht = wpool.tile([128, nh * 128], F16, name="wht")
        nc.sync.dma_start(wht[:].rearrange("k (n m) -> k n m", n=nh),
                          wh_d.rearrange("n k m -> k n m"))

        def wm(key):
            i = index[key]
            if isinstance(i, tuple):
                return wht[:, i[1] * 128:(i[1] + 1) * 128]
            return wft[:, i * 128:(i + 1) * 128]

        def emit_loads(s):
            r0 = _r0(s)
            p_last = H - 1 - r0
            p_lo = 3 if s == 0 else 0
            p_hi = p_last if s == NSLABS - 1 else 127
            m = mpool.tile([128, FW], F32R, name="m", tag="m")
            if s == 0:
                # fill the unused halo partitions with finite data
                nc.gpsimd.dma_start(m[0:2, 1:1 + W], img_d[0, 0:2, :],
                                    accum_op=Al.bypass)
            for c in range(C):
                nc.gpsimd.dma_start(
                    m[p_lo:p_hi + 1, 1:1 + W],
                    img_d[c, r0 + p_lo:r0 + p_hi + 1, :],
                    accum_op=(Al.bypass if c == 0 else Al.add))
            if s == 0:                      # duplicated top edge row at p=2
                for c in range(C):
                    nc.gpsimd.dma_start(
                        m[2:3, 1:1 + W], img_d[c, 0:1, :],
                        accum_op=(Al.bypass if c == 0 else Al.add))
            if s == NSLABS - 1:             # duplicated bottom edge row
                for c in range(C):
                    nc.gpsimd.dma_start(
                        m[p_last + 1:p_last + 2, 1:1 + W],
                        img_d[c, H - 1:H, :],
                        accum_op=(Al.bypass if c == 0 else Al.add))
            # col edge pads (both columns in one strided op, on Pool)
            nc.gpsimd.tensor_copy(m[:, 0:FW:FW - 1], m[:, 1:1 + W:W - 1])
            return m

        def emit_compute(s, m):
            v = 0 if s == 0 else (2 if s == NSLABS - 1 else 1)

            # ---- blur ----
            mb = mbp.tile([128, FW], F32R, name="mb", tag="mb")
            for h in range(2):
                f0 = 1 + CHUNK * h
                pb = ps_b.tile([128, CHUNK], F32, name="pb", tag="pb")
                for i, dc in enumerate(range(3)):
                    nc.tensor.matmul(
                        pb[:], wm(("blur", dc)),
                        m[:, f0 + dc - 1:f0 + dc - 1 + CHUNK],
                        start=(i == 0), stop=(i == 2))
                nc.scalar.activation(mb[:, f0:f0 + CHUNK], pb[:], AF.Copy)
            nc.gpsimd.tensor_copy(mb[:, 0:FW:FW - 1], mb[:, 1:1 + W:W - 1])

            # ---- sobel ----
            pgx = ps_g.tile([128, 2 * CHUNK], F32, name="pgx", tag="pg")
            pgy = ps_g.tile([128, 2 * CHUNK], F32, name="pgy", tag="pg")
            for h in range(2):
                f0 = 1 + CHUNK * h
                w0 = CHUNK * h
                for pt, nm_ in ((pgx, "gx"), (pgy, "gy")):
                    mms = [dc for dc in range(3) if (nm_, v, dc) in index]
                    for i, dc in enumerate(mms):
                        nc.tensor.matmul(
                            pt[:, w0:w0 + CHUNK], wm((nm_, v, dc)),
                            mb[:, f0 + dc - 1:f0 + dc - 1 + CHUNK],
                            start=(i == 0), stop=(i == len(mms) - 1))

            # ---- squares (ACT, fp16) ----
            sxf = sqp.tile([128, W], F16, name="sxf", tag="sxf")
            syf = sqp.tile([128, W], F16, name="syf", tag="syf")
            nc.scalar.activation(sxf[:], pgx[:], AF.Square)
            nc.scalar.activation(syf[:], pgy[:], AF.Square)

            # ---- orientation (recip + arctan + interval test) ----
            rr = tp.tile([128, W], F32, name="rr", tag="rr")
            nc.vector.reciprocal(rr[:], pgx[:])
            tt = tp.tile([128, W], F32, name="tt", tag="tt")
            nc.vector.tensor_tensor(tt[:], rr[:], pgy[:], Al.mult)
            th = tp.tile([128, W], F32, name="th", tag="th")
            nc.scalar.activation(th[:], tt[:], AF.Arctan)
            aa = tp.tile([128, W], F32, name="aa", tag="aa")
            ab = tp.tile([128, W], F32, name="ab", tag="ab")
            nc.vector.tensor_scalar(aa[:], th[:], 8.0 / math.pi, 1.0,
                                    Al.mult, Al.subtract)
            nc.vector.tensor_scalar(ab[:].bitcast(U32), aa[:].bitcast(U32),
                                    0x7FFFFFFF, None, Al.bitwise_and)
            nc.vector.tensor_scalar(aa[:], ab[:], 2.0, None, Al.subtract)
            nc.vector.tensor_scalar(ab[:].bitcast(U32), aa[:].bitcast(U32),
                                    0x7FFFFFFF, None, Al.bitwise_and)
            nmB = tp.tile([128, W], F16, name="nmB", tag="nmB")
            nc.vector.tensor_scalar(nmB[:], ab[:], 0.5, BIGV,
                                    Al.is_ge, Al.mult)

            # ---- M (Pool, fp16) and z = M + B*nm (DVE, fp16) ----
            M = mzp.tile([128, FW], F16, name="M", tag="M")
            nc.gpsimd.tensor_add(M[:, 1:1 + W], sxf[:], syf[:])
            nc.gpsimd.tensor_copy(M[:, 0:FW:FW - 1], M[:, 1:1 + W:W - 1])
            z = mzp.tile([128, W], F16, name="z", tag="z")
            nc.vector.tensor_tensor(z[:], M[:, 1:1 + W], nmB[:], Al.add)

            # ---- NMS shift matmuls (fp32r) + signs (ACT) ----
            s1 = klp.tile([128, W], F16, name="s1", tag="s1")
            s2 = klp.tile([128, W], F16, name="s2", tag="s2")
            v_nms = v
            for h in range(2):
                f0 = 1 + CHUNK * h
                w0 = CHUNK * h
                for st, nm_ in ((s1, "ul"), (s2, "dr")):
                    pn = ps_n.tile([128, CHUNK], F32, name="pn", tag="pn")
                    mms = [dc for dc in range(3) if (nm_, v_nms, dc) in index]
                    for dc in mms:
                        nc.tensor.matmul(
                            pn[:], wm((nm_, v_nms, dc)),
                            M[:, f0 + dc - 1:f0 + dc - 1 + CHUNK],
                            start=(dc == mms[0]), stop=False)
                    nc.tensor.matmul(pn[:], wm(("negI",)),
                                     z[:, w0:w0 + CHUNK],
                                     start=False, stop=True)
                    # keep_side <=> pn < 0 ; Sign(-pn) = +1 there
                    nc.scalar.activation(st[:, w0:w0 + CHUNK], pn[:],
                                         AF.Sign, scale=-1.0)

            # ---- combine + output ----
            ssum = klp.tile([128, W], F16, name="ssum", tag="ssum")
            nc.vector.tensor_tensor(ssum[:], s1[:], s2[:], Al.add)
            kk = klp.tile([128, W], F16, name="kk", tag="kk")
            nc.vector.tensor_scalar(kk[:], ssum[:], 2.0, None, Al.is_equal)
            outv = klp.tile([128, W], F16, name="outv", tag="outv")
            nc.vector.tensor_tensor(outv[:], M[:, 1:1 + W], kk[:], Al.mult)

            row_lo = SLAB * s
            nrows = min(H, row_lo + SLAB) - row_lo
            nc.sync.dma_start(out_d[row_lo:row_lo + nrows, :],
                              outv[3:3 + nrows, :])

        # plain emission (bisect)
        for s in range(NSLABS):
            emit_compute(s, emit_loads(s))

    _split_excess_waits(nc)
    return nc


# ---------------------------------------------------------------------------
def kernel(**inputs):
    _install_fixups()

    img = np.ascontiguousarray(np.asarray(inputs["img"], np.float32))
    gauss_w = np.asarray(inputs["gauss_w"], np.float32)
    sobel_x = np.asarray(inputs["sobel_x"], np.float32)
    sobel_y = np.asarray(inputs["sobel_y"], np.float32)
    dir_w = np.asarray(inputs["dir_w"], np.float32)

    wf, wh, index = _build_bands(gauss_w, sobel_x, sobel_y, dir_w)
    skey = _structure_key(index)
    if _CACHE.get("skey") != skey:
        _CACHE["nc"] = _build_module(index, wf.shape[0], wh.shape[0])
        _CACHE["skey"] = skey
    nc = _CACHE["nc"]

    from concourse.bass_utils import run_bass_kernel_spmd
    import os
    wf = np.ascontiguousarray(wf)
    wh = np.ascontiguousarray(wh)
    in_maps = [{"img": np.ascontiguousarray(img[b]), "wf": wf, "wh": wh}
               for b in range(B)]
    trace = bool(int(os.environ.get("CANNY_TRACE", "0")))
    res = run_bass_kernel_spmd(nc, in_maps, core_ids=list(range(NCORES)),
                               trace=trace)
    if res.exec_time_ns is not None:
        _CACHE["exec_time_ns"] = res.exec_time_ns
    if res.instructions_and_trace is not None:
        _CACHE["trace_path"] = res.instructions_and_trace[1]
    out = np.stack([np.asarray(res.results[b]["out"], np.float32)
                    for b in range(B)])[:, None]
    return out
